# revision 8
# baseline (speedup 1.0000x reference)
"""Trainium2 Bass kernel for nn_EquivariantLayer (spectral equivariant layer).

Strategy (data-parallel over batch, 2 samples/core x 8 cores):
  All FFTs are expressed as real DFT matmuls on the TensorEngine with layouts
  chosen so no corner-turn transposes are ever needed:

    stage1:  A = f^T @ [ExR^T | ExI^T]          (contract x; out [y, (RI,kx)])
    stage2:  F = Ey @ A                          (contract y; out [c, kx], c-major)
             -> two layouts: conv layout [(i%4)*32+c, kx] and fr layout [c, (i,kx)]
    conv:    M = F (*) K elementwise (K = rfft2(sym kernel) is REAL since the
             symmetrized kernel is D4-symmetric); i-reduction via a selector
             matmul on the TensorEngine (PSUM accumulation over i-halves)
    uncurl:  TO_U = i*t, TO_V = i*s are pure-imaginary -> 2 real mults each
    synth:   field = Re(P @ B^T_cm @ Q^T) as two matmul stages (stage a/b)
    cross:   u_a v_b - u_b v_a on the VectorEngine with zero-step broadcast APs

Output [16, 128, 128, 128] f32 (~134 MB) dominates traffic (memory regime).
"""
import sys
import numpy as np

if '/opt/trn_rl_repo' not in sys.path:
    sys.path.insert(0, '/opt/trn_rl_repo')

import concourse.bass as bass
from concourse import bacc
import concourse.mybir as mybir
import concourse.tile as tile
from concourse.bass import AP
from concourse.bass_utils import run_bass_kernel_spmd

F32 = mybir.dt.float32
N_CORES = 8
B_PER_CORE = 2
C1, C2, N1, N2 = 8, 16, 64, 128
NCH_OUT = 128  # 8 fr + 120 cross

I_IDX, J_IDX = np.triu_indices(C2, 1)
_PAIR_IDX = {}
for _p, (_a, _b) in enumerate(zip(I_IDX, J_IDX)):
    _PAIR_IDX[(int(_a), int(_b))] = _p


# ---------------------------------------------------------------------------
# host-side constant construction
# ---------------------------------------------------------------------------

def _host_consts():
    x = np.arange(64)
    kx = np.arange(64)
    c = np.arange(32)
    y = np.arange(64)
    X = np.arange(128)
    Y = np.arange(128)

    FRs = np.where(kx <= 32, kx, kx - 64).astype(np.float64)  # signed row freq

    ExR = np.cos(2 * np.pi * np.outer(kx, x) / 64)   # [kx, x]
    ExI = -np.sin(2 * np.pi * np.outer(kx, x) / 64)
    ExF = np.concatenate([ExR.T, ExI.T], axis=1)     # [x, 128]

    EyR = np.cos(2 * np.pi * np.outer(c, y) / 64)    # [c, y]
    EyI = -np.sin(2 * np.pi * np.outer(c, y) / 64)
    EyRT = EyR.T                                     # [y=64, c=32]
    EyIT = EyI.T
    EyITn = -EyIT

    S_sel = np.zeros((128, 32))
    for im in range(4):
        S_sel[im * 32 + np.arange(32), np.arange(32)] = 1.0

    den = FRs[None, :] ** 2 + c[:, None].astype(np.float64) ** 2
    den[0, 0] = 1.0
    t_u = c[:, None] / den                           # [32, 64]
    s_v = -FRs[None, :] / den
    t_rep = np.tile(t_u, (1, 8))                     # [32, 512] (j-rep)
    s_rep = np.tile(s_v, (1, 8))
    tsg = np.concatenate([-t_rep, t_rep, -s_rep, s_rep], axis=1)  # [32, 2048]

    w_c = np.where(c == 0, 1.0, 2.0)
    s_q = 2.0 / (128.0 * 128.0)
    QRT = (s_q * w_c[None, :] * np.cos(2 * np.pi * np.outer(Y, c) / 128)).T  # [c, Y]
    QIT = (s_q * w_c[None, :] * np.sin(2 * np.pi * np.outer(Y, c) / 128)).T
    QF1 = np.concatenate([QRT, QIT], axis=1)         # [32, 256]
    QF2 = np.concatenate([-QIT, QRT], axis=1)

    PRT = np.cos(2 * np.pi * np.outer(FRs, X) / 128)   # [r=64, X=128]
    PIT = np.sin(2 * np.pi * np.outer(FRs, X) / 128)
    PRT[32, :] = 0.0
    PIT[32, :] = 0.0
    PRT2 = np.concatenate([PRT, PRT], axis=0)        # [128, 128]
    PnIT2 = np.concatenate([-PIT, -PIT], axis=0)

    f32 = lambda a: np.ascontiguousarray(a, dtype=np.float32)
    return dict(ExF=f32(ExF), EyRT=f32(EyRT), EyIT=f32(EyIT), EyITn=f32(EyITn),
                S_sel=f32(S_sel), tsg=f32(tsg), QF1=f32(QF1), QF2=f32(QF2),
                PRT2=f32(PRT2), PnIT2=f32(PnIT2))


def _rot90_kernel(k):
    # z[..., i, j] = k[..., (-j) mod n, i]
    y = np.swapaxes(k, -2, -1)
    return np.concatenate([y[..., :1], y[..., :0:-1]], axis=-1)


def _symmetric_kernel(k):
    k1 = k
    k2 = _rot90_kernel(k1)
    k3 = _rot90_kernel(k2)
    k4 = _rot90_kernel(k3)
    k5 = np.swapaxes(k1, -2, -1)
    k6 = _rot90_kernel(k5)
    k7 = _rot90_kernel(k6)
    k8 = _rot90_kernel(k7)
    return (k1 + k2 + k3 + k4 + k5 + k6 + k7 + k8) / 8.0


def _prep_k_all(kernel_np):
    """kernel [1,8,16,64,64] -> k_all [128, 2048] conv-layout packed."""
    ksym = _symmetric_kernel(kernel_np.astype(np.float64))[0]   # [8,16,64,64]
    K = np.fft.rfft2(ksym).real                                  # [8,16,64,33]
    Kc = np.transpose(K[:, :, :, :32], (0, 1, 3, 2)).copy()      # [i,j,c,kx]
    Kc[:, :, :, 32] = 0.0                                        # kx nyquist
    k_all = np.zeros((128, 2048), dtype=np.float32)
    for i in range(8):
        h, im = i // 4, i % 4
        for j in range(16):
            k_all[im * 32:(im + 1) * 32, j * 128 + h * 64: j * 128 + h * 64 + 64] = Kc[i, j]
    return k_all


# ---------------------------------------------------------------------------
# device program
# ---------------------------------------------------------------------------

def _bcast(ap, n, axis_pos=1):
    """Insert a zero-step broadcast dim of size n into an AP (after partition dim)."""
    dims = list(ap.ap)
    dims.insert(axis_pos, [0, n])
    return AP(ap.tensor, ap.offset, dims)


def _view(ap, offset_elems, dims):
    """Raw AP view on the same tensor: explicit offset (elems) + [step, count] dims."""
    return AP(ap.tensor, ap.offset + offset_elems, dims)


def build_program():
    nc = bacc.Bacc("TRN2", target_bir_lowering=False)
    consts = _host_consts()

    f_in = nc.dram_tensor("f_in", [B_PER_CORE, C1, 64, 64], F32, kind="ExternalInput")
    k_in = nc.dram_tensor("k_all", [128, 2048], F32, kind="ExternalInput")
    out_sh = nc.dram_tensor("out_sh", [B_PER_CORE, NCH_OUT, 128, 128], F32,
                            kind="ExternalOutput")

    cdr = {name: nc.inline_tensor(arr, name=f"c_{name}") for name, arr in consts.items()}

    with tile.TileContext(nc) as tc:
        with (
            tc.tile_pool(name="cp", bufs=1) as cp,
            tc.tile_pool(name="fld", bufs=1) as fld,     # u_all/v_all/fr_all
            tc.tile_pool(name="wk", bufs=2) as wk,       # small working tiles
            tc.tile_pool(name="mw", bufs=1) as mwp,      # conv wide tiles
            tc.tile_pool(name="wp", bufs=1) as wp,       # cross product blocks
            tc.tile_pool(name="crp", bufs=2) as crp,     # cross output staging
            tc.tile_pool(name="pp", bufs=1, space="PSUM") as pp,
        ):
            # ---- load constants ----
            cs = {}
            for name, arr in consts.items():
                t = cp.tile(list(arr.shape), F32, tag=f"c_{name}", name=f"cs_{name}")
                nc.sync.dma_start(out=t[:], in_=cdr[name][:])
                cs[name] = t
            k_sb = cp.tile([128, 2048], F32, tag="k_sb")
            nc.sync.dma_start(out=k_sb[:], in_=k_in[:])

            u_all = fld.tile([128, 16 * 256], F32, tag="u_all")
            v_all = fld.tile([128, 16 * 256], F32, tag="v_all")
            fr_all = fld.tile([128, 8 * 256], F32, tag="fr_all")

            for b in range(B_PER_CORE):
                # ================= stage 1: x-DFT =================
                A_ch = []
                for ip in range(4):
                    fsb = wk.tile([64, 128], F32, tag="fsb")
                    nc.sync.dma_start(
                        out=fsb[:].rearrange("x (i y) -> x i y", i=2),
                        in_=f_in[b, 2 * ip:2 * ip + 2].rearrange("i x y -> x i y"))
                    psA = pp.tile([128, 128], F32, tag="bankA", bufs=2)
                    nc.tensor.matmul(psA[:], fsb[:], cs["ExF"][:], start=True, stop=True)
                    for iloc in range(2):
                        a_t = wk.tile([64, 128], F32, tag=f"ach{2*ip+iloc}")
                        nc.scalar.copy(out=a_t[:], in_=psA[iloc * 64:(iloc + 1) * 64, :])
                        A_ch.append(a_t)

                # ================= stage 2: y-DFT =================
                psFcvR = [pp.tile([128, 64], F32, tag=f"bankF{h}", name=f"psFcvR{h}") for h in range(2)]
                psFcvI = [pp.tile([128, 64], F32, tag=f"bankF{2+h}", name=f"psFcvI{h}") for h in range(2)]
                psFfrR = pp.tile([32, 512], F32, tag="bankF4")
                psFfrI = pp.tile([32, 512], F32, tag="bankF5")
                EyR, EyI, EyIn = cs["EyRT"], cs["EyIT"], cs["EyITn"]
                for i in range(8):
                    A_R = A_ch[i][:, 0:64]
                    A_I = A_ch[i][:, 64:128]
                    h, im = i // 4, i % 4
                    sl = slice(im * 32, (im + 1) * 32)
                    tp = (0, im * 32)
                    nc.tensor.matmul(psFcvR[h][sl, :], EyR[:], A_R, start=True, stop=False,
                                     tile_position=tp)
                    nc.tensor.matmul(psFcvR[h][sl, :], EyIn[:], A_I, start=False, stop=True,
                                     tile_position=tp)
                    nc.tensor.matmul(psFcvI[h][sl, :], EyI[:], A_R, start=True, stop=False,
                                     tile_position=tp)
                    nc.tensor.matmul(psFcvI[h][sl, :], EyR[:], A_I, start=False, stop=True,
                                     tile_position=tp)
                    fsl = slice(i * 64, (i + 1) * 64)
                    nc.tensor.matmul(psFfrR[:, fsl], EyR[:], A_R, start=True, stop=False)
                    nc.tensor.matmul(psFfrR[:, fsl], EyIn[:], A_I, start=False, stop=True)
                    nc.tensor.matmul(psFfrI[:, fsl], EyI[:], A_R, start=True, stop=False)
                    nc.tensor.matmul(psFfrI[:, fsl], EyR[:], A_I, start=False, stop=True)

                Fcv = wk.tile([128, 256], F32, tag="Fcv")
                for h in range(2):
                    nc.scalar.copy(out=Fcv[:, h * 64:(h + 1) * 64], in_=psFcvR[h][:])
                    nc.scalar.copy(out=Fcv[:, 128 + h * 64:128 + (h + 1) * 64], in_=psFcvI[h][:])
                Ffr = wk.tile([32, 1024], F32, tag="Ffr")
                nc.scalar.copy(out=Ffr[:, 0:512], in_=psFfrR[:])
                nc.scalar.copy(out=Ffr[:, 512:1024], in_=psFfrI[:])

                # ================= conv: M = F (*) K =================
                Mw = []
                for RI in range(2):
                    m_t = mwp.tile([128, 2048], F32, tag=f"mw{RI}")
                    in0 = _bcast(Fcv[:, RI * 128:(RI + 1) * 128], 16)
                    nc.vector.tensor_mul(
                        m_t[:].rearrange("p (j f) -> p j f", j=16),
                        in0,
                        k_sb[:].rearrange("p (j f) -> p j f", j=16))
                    Mw.append(m_t)

                # ============ conv-reduce (PE) + uncurl (DVE) ============
                BuR = wk.tile([32, 1024], F32, tag="BuR")
                BuI = wk.tile([32, 1024], F32, tag="BuI")
                BvR = wk.tile([32, 1024], F32, tag="BvR")
                BvI = wk.tile([32, 1024], F32, tag="BvI")
                tsg = cs["tsg"]
                for RI in range(2):
                    for jh in range(2):
                        ps_acv = pp.tile([32, 512], F32, tag="bankA", bufs=2)
                        for h in range(2):
                            rhs = _view(Mw[RI][:], jh * 1024 + h * 64,
                                        [Mw[RI][:].ap[0], [128, 8], [1, 64]])
                            nc.tensor.matmul(ps_acv[:], cs["S_sel"][:], rhs,
                                             start=(h == 0), stop=(h == 1))
                        osl = slice(jh * 512, (jh + 1) * 512)
                        if RI == 0:  # A_R -> imaginary parts of Bu/Bv
                            nc.vector.tensor_mul(BuI[:, osl], ps_acv[:], tsg[:, 512:1024])
                            nc.vector.tensor_mul(BvI[:, osl], ps_acv[:], tsg[:, 1536:2048])
                        else:        # A_I -> real parts (negated multipliers)
                            nc.vector.tensor_mul(BuR[:, osl], ps_acv[:], tsg[:, 0:512])
                            nc.vector.tensor_mul(BvR[:, osl], ps_acv[:], tsg[:, 1024:1536])

                # ============ synthesis: stage a (Q) + stage b (P) ============
                # fields: (B_R source, B_I source, dest tile, #channels, dma out?)
                fields = [
                    (BuR[:], BuI[:], u_all, 16, None),
                    (BvR[:], BvI[:], v_all, 16, None),
                    (Ffr[:, 0:512], Ffr[:, 512:1024], fr_all, 8, True),
                ]
                for BR, BI, dest, nch, is_fr in fields:
                    for cpair in range(nch // 2):
                        csl = slice(cpair * 128, (cpair + 1) * 128)
                        psG = pp.tile([128, 256], F32, tag=f"bankF{cpair % 2}", name="psG")
                        nc.tensor.matmul(psG[:], BR[:, csl], cs["QF1"][:], start=True, stop=False)
                        nc.tensor.matmul(psG[:], BI[:, csl], cs["QF2"][:], start=False, stop=True)
                        G_sb = wk.tile([128, 256], F32, tag="G_sb")
                        nc.scalar.copy(out=G_sb[:], in_=psG[:])
                        for cl in range(2):
                            ch = 2 * cpair + cl
                            rsl = slice(cl * 64, (cl + 1) * 64)
                            psU = pp.tile([128, 128], F32, tag=f"bankF{2 + (2 * cpair + cl) % 2}", name="psU")
                            nc.tensor.matmul(psU[:], cs["PRT2"][rsl, :], G_sb[rsl, 0:128],
                                             start=True, stop=False)
                            nc.tensor.matmul(psU[:], cs["PnIT2"][rsl, :], G_sb[rsl, 128:256],
                                             start=False, stop=True)
                            dsl = slice(ch * 256 + b * 128, ch * 256 + (b + 1) * 128)
                            nc.scalar.copy(out=dest[:, dsl], in_=psU[:])
                            if is_fr:
                                nc.sync.dma_start(out=out_sh[b, ch], in_=dest[:, dsl])

            # ================= cross products =================
            for gI in range(4):
                for gJ in range(gI, 4):
                    W1 = wp.tile([128, 4096], F32, tag="W1")
                    for ai in range(4):
                        a = 4 * gI + ai
                        in0 = _bcast(u_all[:, a * 256:(a + 1) * 256], 4)
                        in1 = v_all[:, gJ * 1024:(gJ + 1) * 1024].rearrange(
                            "p (cb f) -> p cb f", cb=4)
                        nc.vector.tensor_mul(
                            W1[:, ai * 1024:(ai + 1) * 1024].rearrange(
                                "p (cb f) -> p cb f", cb=4), in0, in1)
                    if gI != gJ:
                        W2 = wp.tile([128, 4096], F32, tag="W2")
                        for bjl in range(4):
                            bj = 4 * gJ + bjl
                            in0 = _bcast(u_all[:, bj * 256:(bj + 1) * 256], 4)
                            in1 = v_all[:, gI * 1024:(gI + 1) * 1024].rearrange(
                                "p (ca f) -> p ca f", ca=4)
                            nc.vector.tensor_mul(
                                W2[:, bjl * 1024:(bjl + 1) * 1024].rearrange(
                                    "p (ca f) -> p ca f", ca=4), in0, in1)
                        for ai in range(4):
                            a = 4 * gI + ai
                            cr = crp.tile([128, 1024], F32, tag="cr")
                            in0 = W1[:, ai * 1024:(ai + 1) * 1024].rearrange(
                                "p (cb f) -> p cb f", cb=4)
                            in1 = _view(W2[:], ai * 256,
                                        [W2[:].ap[0], [1024, 4], [1, 256]])
                            nc.vector.tensor_sub(
                                cr[:].rearrange("p (cb f) -> p cb f", cb=4), in0, in1)
                            for bjl in range(4):
                                bj = 4 * gJ + bjl
                                pch = 8 + _PAIR_IDX[(a, bj)]
                                for bs in range(B_PER_CORE):
                                    nc.sync.dma_start(
                                        out=out_sh[bs, pch],
                                        in_=cr[:, bjl * 256 + bs * 128:
                                               bjl * 256 + (bs + 1) * 128])
                    else:
                        for ai in range(3):
                            a = 4 * gI + ai
                            cnt = 3 - ai
                            cr = crp.tile([128, 1024], F32, tag="cr")
                            in0 = W1[:, ai * 1024 + (ai + 1) * 256:(ai + 1) * 1024]
                            in0 = AP(in0.tensor, in0.offset,
                                     [in0.ap[0], [256, cnt], [1, 256]])
                            in1 = _view(W1[:], (ai + 1) * 1024 + ai * 256,
                                        [W1[:].ap[0], [1024, cnt], [1, 256]])
                            nc.vector.tensor_sub(
                                cr[:, 0:cnt * 256].rearrange(
                                    "p (cb f) -> p cb f", cb=cnt), in0, in1)
                            for k in range(cnt):
                                bj = a + 1 + k
                                pch = 8 + _PAIR_IDX[(a, bj)]
                                for bs in range(B_PER_CORE):
                                    nc.sync.dma_start(
                                        out=out_sh[bs, pch],
                                        in_=cr[:, k * 256 + bs * 128:
                                               k * 256 + (bs + 1) * 128])
    nc.compile()
    return nc


# ---------------------------------------------------------------------------
# entry point
# ---------------------------------------------------------------------------

_PROGRAM = None


def _get_program():
    global _PROGRAM
    if _PROGRAM is None:
        _PROGRAM = build_program()
    return _PROGRAM


LAST_EXEC_NS = None
LAST_RESULT = None


def kernel(f, kernel):
    global LAST_EXEC_NS, LAST_RESULT
    f = np.ascontiguousarray(f, dtype=np.float32)
    k_all = _prep_k_all(np.asarray(kernel))
    nc = _get_program()
    in_maps = [
        {"f_in": f[2 * c:2 * c + 2], "k_all": k_all} for c in range(N_CORES)
    ]
    import os
    trace = bool(os.environ.get("KERNEL_TRACE"))
    res = run_bass_kernel_spmd(nc, in_maps, list(range(N_CORES)), trace=trace)
    LAST_RESULT = res
    if res.exec_time_ns is not None:
        LAST_EXEC_NS = res.exec_time_ns
    out = np.concatenate([res.results[c]["out_sh"] for c in range(N_CORES)], axis=0)
    return out
